# revision 5
# baseline (speedup 1.0000x reference)
"""MHA kernel for Trainium2, 8 NeuronCores (SPMD, no collectives).

Problem: B=4, L=2048, DMODEL=1024, 16 heads x 64. out = MHA(q,k,v; Wq,Wk,Wv).

Sharding: core c handles batch b=c//2 and heads h0=(c%2)*8 .. h0+8.
Each core gets x = q/k/v[b] pre-transposed to [DMODEL, L] (k-major) and its
8-head weight slices pre-transposed to [DMODEL, 512]; it produces
outT [512, L] = (attn @ V)^T unnormalized plus sums [8, L] (softmax
denominators); the host divides, transposes and concatenates.

On-chip layout trick: everything stays "contraction-major" so no transposes
are ever needed:
  qh_T/kh_T [feat, tok] come from matmul(lhsT=W_t, rhs=x_t)
  vh        [tok, feat] comes from matmul(lhsT=x_t, rhs=W_t)
  scores_T  [tk, tq]    = matmul(lhsT=kh_T, rhs=qh_T)   (K=64)
  exp_T     [tk, tq]    = ACT exp on scores_T
  out_T     [65, tq]    = matmul(lhsT=vh_aug, rhs=exp_T) accumulated over tk,
                          where vh_aug has a ones column per head -> row 64
                          of out_T is the softmax denominator.
"""

import sys

try:
    import concourse  # noqa: F401
except ImportError:  # pragma: no cover
    sys.path.insert(0, "/opt/trn_rl_repo")

import numpy as np
import concourse.mybir as mybir
import concourse.tile as tile
from concourse import bacc
from concourse.bass_utils import run_bass_kernel_spmd

B = 4
L = 2048
DM = 1024
NH = 16
D = 64
NCORES = 8
H = 8          # heads per core
F = H * D      # 512 features per core
P = 128
KT = DM // P   # 8 contraction tiles for projections
LT = L // P    # 16 token tiles
NTQC = 4       # 512-wide tq chunks
TQH = 1024     # tq half processed per inner phase-2 block

f32 = mybir.dt.float32
f32r = mybir.dt.float32r

MM_DT = f32    # matmul input dtype: f32 (exact) or f32r (tf32-like, 4x faster)


def build_program(mm_dt=MM_DT):
    nc = bacc.Bacc("TRN2", target_bir_lowering=False, debug=False)

    xq = nc.dram_tensor("xq", [DM, L], f32, kind="ExternalInput")
    xk = nc.dram_tensor("xk", [DM, L], f32, kind="ExternalInput")
    xv = nc.dram_tensor("xv", [DM, L], f32, kind="ExternalInput")
    wq = nc.dram_tensor("wq", [DM, F], f32, kind="ExternalInput")
    wk = nc.dram_tensor("wk", [DM, F], f32, kind="ExternalInput")
    wv = nc.dram_tensor("wv", [DM, F], f32, kind="ExternalInput")
    outT = nc.dram_tensor("outT", [F, L], f32, kind="ExternalOutput")
    sums = nc.dram_tensor("sums", [H, L], f32, kind="ExternalOutput")

    Exp = mybir.ActivationFunctionType.Exp

    with tile.TileContext(nc) as tc:
        with tc.tile_pool(name="persist", bufs=1) as persist:
            # [feat, tok] per j-tile (2 heads each); [tok, 8*(64+1)] for v
            qh = [persist.tile([P, L], mm_dt, tag=f"qh{i}", name=f"qh{i}") for i in range(4)]
            kh = [persist.tile([P, L], mm_dt, tag=f"kh{i}", name=f"kh{i}") for i in range(4)]
            vh = [persist.tile([P, H * (D + 1)], mm_dt, tag=f"vh{i}", name=f"vh{i}") for i in range(LT)]

            # ---------------- phase 1: projections ----------------
            with (
                tc.tile_pool(name="wpool", bufs=1) as wpool,
                tc.tile_pool(name="xpool", bufs=1) as xpool,
                tc.tile_pool(name="ppool", bufs=4, space="PSUM") as ppool,
            ):
                rounded = mm_dt != f32

                w_sb = {}
                for wname, wdram in (("q", wq), ("k", wk), ("v", wv)):
                    for kk in range(KT):
                        t = wpool.tile([P, F], mm_dt, tag="w", bufs=26, name=f"w_{wname}{kk}")
                        if rounded:
                            stg = wpool.tile([P, F], f32, tag="stg", bufs=4, name=f"wstg_{wname}{kk}")
                            nc.sync.dma_start(stg[:], wdram[kk * P : (kk + 1) * P, :])
                            nc.vector.tensor_copy(t[:], stg[:])
                        else:
                            nc.sync.dma_start(t[:], wdram[kk * P : (kk + 1) * P, :])
                        w_sb[wname, kk] = t

                # Q/K projections: out_T[j, t] accumulated over k
                for xname, xdram, dst in (("q", xq, qh), ("k", xk, kh)):
                    for tt in range(NTQC):
                        x_ch = []
                        for kk in range(KT):
                            t = xpool.tile([P, 512], mm_dt, tag="x", bufs=(20 if rounded else 24), name=f"x_{xname}{tt}_{kk}")
                            if rounded:
                                stg = xpool.tile([P, 512], f32, tag="stg", bufs=4, name=f"xstg_{xname}{tt}_{kk}")
                                nc.sync.dma_start(stg[:], xdram[kk * P : (kk + 1) * P, tt * 512 : (tt + 1) * 512])
                                nc.vector.tensor_copy(t[:], stg[:])
                            else:
                                nc.sync.dma_start(t[:], xdram[kk * P : (kk + 1) * P, tt * 512 : (tt + 1) * 512])
                            x_ch.append(t)
                        for jt in range(4):
                            ps = ppool.tile([P, 512], f32, tag="pp", bufs=4, name=f"ps_{xname}{tt}_{jt}")
                            for kk in range(KT):
                                nc.tensor.matmul(
                                    ps[:],
                                    w_sb[xname, kk][:, jt * P : (jt + 1) * P],
                                    x_ch[kk][:],
                                    start=(kk == 0),
                                    stop=(kk == KT - 1),
                                )
                            eng = nc.vector if xname == "q" else nc.scalar
                            if xname == "q":
                                eng.tensor_copy(dst[jt][:, tt * 512 : (tt + 1) * 512], ps[:])
                            else:
                                eng.copy(dst[jt][:, tt * 512 : (tt + 1) * 512], ps[:])

                # V projection: vh[t, j] with ones column per head
                for tt in range(NTQC):
                    x_ch = []
                    for kk in range(KT):
                        t = xpool.tile([P, 512], mm_dt, tag="x", bufs=(20 if rounded else 24), name=f"x_v{tt}_{kk}")
                        if rounded:
                            stg = xpool.tile([P, 512], f32, tag="stg", bufs=4, name=f"xstg_v{tt}_{kk}")
                            nc.sync.dma_start(stg[:], xv[kk * P : (kk + 1) * P, tt * 512 : (tt + 1) * 512])
                            nc.vector.tensor_copy(t[:], stg[:])
                        else:
                            nc.sync.dma_start(t[:], xv[kk * P : (kk + 1) * P, tt * 512 : (tt + 1) * 512])
                        x_ch.append(t)
                    for sub in range(4):
                        ti = tt * 4 + sub  # token tile index 0..15
                        ps = ppool.tile([P, F], f32, tag="pp", bufs=4, name=f"ps_v{ti}")
                        for kk in range(KT):
                            nc.tensor.matmul(
                                ps[:],
                                x_ch[kk][:, sub * P : (sub + 1) * P],
                                w_sb["v", kk][:],
                                start=(kk == 0),
                                stop=(kk == KT - 1),
                            )
                        vt = vh[ti]
                        ones_col = vt[:].rearrange("p (h e) -> p h e", e=D + 1)[:, :, D]
                        nc.vector.memset(ones_col, 1.0)
                        nc.vector.tensor_copy(
                            vt[:].rearrange("p (h e) -> p h e", e=D + 1)[:, :, 0:D],
                            ps[:].rearrange("p (h e) -> p h e", e=D),
                        )

            # ---------------- phase 2: attention ----------------
            with (
                tc.tile_pool(name="spool", bufs=1, space="PSUM") as spool,
                tc.tile_pool(name="opool", bufs=1, space="PSUM") as opool,
                tc.tile_pool(name="epool", bufs=1) as epool,
                tc.tile_pool(name="apool", bufs=1) as apool,
            ):
                for p in range(4):  # head pair: heads 2p (rows 0:64), 2p+1 (rows 64:128)
                    av = []
                    for hh in range(2):
                        head = 2 * p + hh
                        t = apool.tile([D + 1, L], f32, tag="av", bufs=3, name=f"av{head}")
                        av.append(t)
                    for hf in range(2):  # tq half
                        po = {}
                        for hh in range(2):
                            for c in range(2):
                                po[hh, c] = opool.tile(
                                    [D + 1, 512], f32, tag="po", bufs=4, name=f"po{p}_{hf}_{hh}_{c}"
                                )
                        for tk in range(LT):
                            es = []
                            for hh in range(2):
                                r = hh * D
                                sc = spool.tile([P, TQH], f32, tag="sc", bufs=2, name=f"sc{p}_{hf}_{tk}_{hh}")
                                for c in range(2):
                                    nc.tensor.matmul(
                                        sc[:, c * 512 : (c + 1) * 512],
                                        kh[p][r : r + D, tk * P : (tk + 1) * P],
                                        qh[p][r : r + D, hf * TQH + c * 512 : hf * TQH + (c + 1) * 512],
                                        start=True,
                                        stop=True,
                                    )
                                e = epool.tile([P, TQH], mm_dt, tag="exp", bufs=4, name=f"e{p}_{hf}_{tk}_{hh}")
                                nc.scalar.activation(e[:], sc[:], Exp)
                                es.append(e)
                            for hh in range(2):
                                head = 2 * p + hh
                                for c in range(2):
                                    nc.tensor.matmul(
                                        po[hh, c][:],
                                        vh[tk][:, head * (D + 1) : (head + 1) * (D + 1)],
                                        es[hh][:, c * 512 : (c + 1) * 512],
                                        start=(tk == 0),
                                        stop=(tk == LT - 1),
                                    )
                        for hh in range(2):
                            for c in range(2):
                                lo = hf * TQH + c * 512
                                nc.vector.tensor_copy(av[hh][:, lo : lo + 512], po[hh, c][:])
                    for hh in range(2):
                        head = 2 * p + hh
                        nc.sync.dma_start(outT[head * D : (head + 1) * D, :], av[hh][0:D, :])
                        nc.sync.dma_start(sums[head : head + 1, :], av[hh][D : D + 1, :])

    nc.compile()
    return nc


_program_cache = {}


def _get_program(mm_dt=MM_DT):
    key = str(mm_dt)
    if key not in _program_cache:
        _program_cache[key] = build_program(mm_dt)
    return _program_cache[key]


def _make_in_maps(q, k, v, Wq, Wk, Wv):
    scale = 1.0 / np.sqrt(np.float32(D))
    q = np.asarray(q, dtype=np.float32)
    k = np.asarray(k, dtype=np.float32)
    v = np.asarray(v, dtype=np.float32)
    xqs = [np.ascontiguousarray(q[b].T) for b in range(B)]
    xks = [np.ascontiguousarray(k[b].T) for b in range(B)]
    xvs = [np.ascontiguousarray(v[b].T) for b in range(B)]
    in_maps = []
    for c in range(NCORES):
        b = c // 2
        h0 = (c % 2) * H
        rows = slice(h0 * D, (h0 + H) * D)
        in_maps.append(
            {
                "xq": xqs[b],
                "xk": xks[b],
                "xv": xvs[b],
                "wq": np.ascontiguousarray(np.asarray(Wq, np.float32)[rows, :].T * scale),
                "wk": np.ascontiguousarray(np.asarray(Wk, np.float32)[rows, :].T),
                "wv": np.ascontiguousarray(np.asarray(Wv, np.float32)[rows, :].T),
            }
        )
    return in_maps


def _assemble(results):
    out = np.empty((B, L, NH * D), dtype=np.float32)
    for c in range(NCORES):
        b = c // 2
        h0 = (c % 2) * H
        r = results[c]
        unnorm = r["outT"].reshape(H, D, L)
        denom = r["sums"].reshape(H, 1, L)
        core_out = (unnorm / denom).transpose(2, 0, 1).reshape(L, F)
        out[b, :, h0 * D : (h0 + H) * D] = core_out
    return out


def run_mha(q, k, v, Wq, Wk, Wv, trace=False, mm_dt=MM_DT, **run_kwargs):
    nc = _get_program(mm_dt)
    in_maps = _make_in_maps(q, k, v, Wq, Wk, Wv)
    res = run_bass_kernel_spmd(nc, in_maps, core_ids=list(range(NCORES)), trace=trace, **run_kwargs)
    return _assemble(res.results), res


def kernel(q, k, v, Wq, Wk, Wv):
    out, _ = run_mha(q, k, v, Wq, Wk, Wv)
    return out


# revision 7
# speedup vs baseline: 2.9962x; 2.9962x over previous
"""MHA kernel for Trainium2, 8 NeuronCores (SPMD, no collectives).

Problem: B=4, L=2048, DMODEL=1024, 16 heads x 64. out = MHA(q,k,v; Wq,Wk,Wv).

Sharding: core c handles batch b=c//2 and heads h0=(c%2)*8 .. h0+8.
Each core gets x = q/k/v[b] pre-transposed to [DMODEL, L] (k-major) and its
8-head weight slices pre-transposed to [DMODEL, 512]; it produces
outT [512, L] = (attn @ V)^T unnormalized plus sums [8, L] (softmax
denominators); the host divides, transposes and concatenates.

On-chip layout trick: everything stays "contraction-major" so no transposes
are ever needed:
  qh_T/kh_T [feat, tok] come from matmul(lhsT=W_t, rhs=x_t)
  vh        [tok, feat] comes from matmul(lhsT=x_t, rhs=W_t)
  scores_T  [tk, tq]    = matmul(lhsT=kh_T, rhs=qh_T)   (K=64)
  exp_T     [tk, tq]    = ACT exp on scores_T
  out_T     [65, tq]    = matmul(lhsT=vh_aug, rhs=exp_T) accumulated over tk,
                          where vh_aug has a ones column per head -> row 64
                          of out_T is the softmax denominator.
"""

import sys

try:
    import concourse  # noqa: F401
except ImportError:  # pragma: no cover
    sys.path.insert(0, "/opt/trn_rl_repo")

import numpy as np
import concourse.mybir as mybir
import concourse.tile as tile
from concourse import bacc
from concourse.bass_utils import run_bass_kernel_spmd

B = 4
L = 2048
DM = 1024
NH = 16
D = 64
NCORES = 8
H = 8          # heads per core
F = H * D      # 512 features per core
P = 128
KT = DM // P   # 8 contraction tiles for projections
LT = L // P    # 16 token tiles
NTQC = 4       # 512-wide tq chunks
TQH = 1024     # tq half processed per inner phase-2 block

f32 = mybir.dt.float32
f32r = mybir.dt.float32r

MM_DT = f32    # matmul input dtype: f32 (exact) or f32r (tf32-like, 4x faster)


def build_program(mm_dt=MM_DT):
    nc = bacc.Bacc("TRN2", target_bir_lowering=False, debug=False)

    xq = nc.dram_tensor("xq", [DM, L], f32, kind="ExternalInput")
    xk = nc.dram_tensor("xk", [DM, L], f32, kind="ExternalInput")
    xv = nc.dram_tensor("xv", [DM, L], f32, kind="ExternalInput")
    wq = nc.dram_tensor("wq", [DM, F], f32, kind="ExternalInput")
    wk = nc.dram_tensor("wk", [DM, F], f32, kind="ExternalInput")
    wv = nc.dram_tensor("wv", [DM, F], f32, kind="ExternalInput")
    outT = nc.dram_tensor("outT", [F, L], f32, kind="ExternalOutput")
    sums = nc.dram_tensor("sums", [H, L], f32, kind="ExternalOutput")

    Exp = mybir.ActivationFunctionType.Exp

    with tile.TileContext(nc) as tc:
        with tc.tile_pool(name="persist", bufs=1) as persist:
            # [feat, tok] per j-tile (2 heads each); [tok, 8*(64+1)] for v
            qh = [persist.tile([P, L], mm_dt, tag=f"qh{i}", name=f"qh{i}") for i in range(4)]
            kh = [persist.tile([P, L], mm_dt, tag=f"kh{i}", name=f"kh{i}") for i in range(4)]
            vh = [persist.tile([P, H * (D + 1)], mm_dt, tag=f"vh{i}", name=f"vh{i}") for i in range(LT)]

            # ---------------- phase 1: projections ----------------
            with (
                tc.tile_pool(name="wpool", bufs=1) as wpool,
                tc.tile_pool(name="xpool", bufs=1) as xpool,
                tc.tile_pool(name="ppool", bufs=4, space="PSUM") as ppool,
            ):
                rounded = mm_dt != f32

                w_sb = {}
                for wname, wdram in (("q", wq), ("k", wk), ("v", wv)):
                    for kk in range(KT):
                        t = wpool.tile([P, F], mm_dt, tag="w", bufs=26, name=f"w_{wname}{kk}")
                        if rounded:
                            stg = wpool.tile([P, F], f32, tag="stg", bufs=4, name=f"wstg_{wname}{kk}")
                            nc.sync.dma_start(stg[:], wdram[kk * P : (kk + 1) * P, :])
                            nc.vector.tensor_copy(t[:], stg[:])
                        else:
                            nc.sync.dma_start(t[:], wdram[kk * P : (kk + 1) * P, :])
                        w_sb[wname, kk] = t

                # Q/K projections: out_T[j, t] accumulated over k
                for xname, xdram, dst in (("q", xq, qh), ("k", xk, kh)):
                    for tt in range(NTQC):
                        x_ch = []
                        for kk in range(KT):
                            t = xpool.tile([P, 512], mm_dt, tag="x", bufs=(20 if rounded else 24), name=f"x_{xname}{tt}_{kk}")
                            if rounded:
                                stg = xpool.tile([P, 512], f32, tag="stg", bufs=4, name=f"xstg_{xname}{tt}_{kk}")
                                nc.sync.dma_start(stg[:], xdram[kk * P : (kk + 1) * P, tt * 512 : (tt + 1) * 512])
                                nc.vector.tensor_copy(t[:], stg[:])
                            else:
                                nc.sync.dma_start(t[:], xdram[kk * P : (kk + 1) * P, tt * 512 : (tt + 1) * 512])
                            x_ch.append(t)
                        for jt in range(4):
                            ps = ppool.tile([P, 512], f32, tag="pp", bufs=4, name=f"ps_{xname}{tt}_{jt}")
                            for kk in range(KT):
                                nc.tensor.matmul(
                                    ps[:],
                                    w_sb[xname, kk][:, jt * P : (jt + 1) * P],
                                    x_ch[kk][:],
                                    start=(kk == 0),
                                    stop=(kk == KT - 1),
                                )
                            eng = nc.vector if xname == "q" else nc.scalar
                            if xname == "q":
                                eng.tensor_copy(dst[jt][:, tt * 512 : (tt + 1) * 512], ps[:])
                            else:
                                eng.copy(dst[jt][:, tt * 512 : (tt + 1) * 512], ps[:])

                # V projection: vh[t, j] with ones column per head
                if rounded:
                    ones_f32 = wpool.tile([P, H], f32, tag="ones", name="ones_f32")
                    nc.vector.memset(ones_f32[:], 1.0)
                for tt in range(NTQC):
                    x_ch = []
                    for kk in range(KT):
                        t = xpool.tile([P, 512], mm_dt, tag="x", bufs=(20 if rounded else 24), name=f"x_v{tt}_{kk}")
                        if rounded:
                            stg = xpool.tile([P, 512], f32, tag="stg", bufs=4, name=f"xstg_v{tt}_{kk}")
                            nc.sync.dma_start(stg[:], xv[kk * P : (kk + 1) * P, tt * 512 : (tt + 1) * 512])
                            nc.vector.tensor_copy(t[:], stg[:])
                        else:
                            nc.sync.dma_start(t[:], xv[kk * P : (kk + 1) * P, tt * 512 : (tt + 1) * 512])
                        x_ch.append(t)
                    for sub in range(4):
                        ti = tt * 4 + sub  # token tile index 0..15
                        ps = ppool.tile([P, F], f32, tag="pp", bufs=4, name=f"ps_v{ti}")
                        for kk in range(KT):
                            nc.tensor.matmul(
                                ps[:],
                                x_ch[kk][:, sub * P : (sub + 1) * P],
                                w_sb["v", kk][:],
                                start=(kk == 0),
                                stop=(kk == KT - 1),
                            )
                        vt = vh[ti]
                        ones_col = vt[:].rearrange("p (h e) -> p h e", e=D + 1)[:, :, D]
                        if rounded:
                            nc.vector.tensor_copy(ones_col, ones_f32[:])
                        else:
                            nc.vector.memset(ones_col, 1.0)
                        nc.vector.tensor_copy(
                            vt[:].rearrange("p (h e) -> p h e", e=D + 1)[:, :, 0:D],
                            ps[:].rearrange("p (h e) -> p h e", e=D),
                        )

            # ---------------- phase 2: attention ----------------
            with (
                tc.tile_pool(name="spool", bufs=1, space="PSUM") as spool,
                tc.tile_pool(name="opool", bufs=1, space="PSUM") as opool,
                tc.tile_pool(name="epool", bufs=1) as epool,
                tc.tile_pool(name="apool", bufs=1) as apool,
            ):
                for p in range(4):  # head pair: heads 2p (rows 0:64), 2p+1 (rows 64:128)
                    av = []
                    for hh in range(2):
                        head = 2 * p + hh
                        t = apool.tile([D + 1, L], f32, tag="av", bufs=3, name=f"av{head}")
                        av.append(t)
                    for hf in range(2):  # tq half
                        po = {}
                        for hh in range(2):
                            for c in range(2):
                                po[hh, c] = opool.tile(
                                    [D + 1, 512], f32, tag="po", bufs=4, name=f"po{p}_{hf}_{hh}_{c}"
                                )
                        for tk in range(LT):
                            es = []
                            for hh in range(2):
                                r = hh * D
                                sc = spool.tile([P, TQH], f32, tag="sc", bufs=2, name=f"sc{p}_{hf}_{tk}_{hh}")
                                for c in range(2):
                                    nc.tensor.matmul(
                                        sc[:, c * 512 : (c + 1) * 512],
                                        kh[p][r : r + D, tk * P : (tk + 1) * P],
                                        qh[p][r : r + D, hf * TQH + c * 512 : hf * TQH + (c + 1) * 512],
                                        start=True,
                                        stop=True,
                                    )
                                e = epool.tile([P, TQH], mm_dt, tag="exp", bufs=4, name=f"e{p}_{hf}_{tk}_{hh}")
                                nc.scalar.activation(e[:], sc[:], Exp)
                                es.append(e)
                            for hh in range(2):
                                head = 2 * p + hh
                                for c in range(2):
                                    nc.tensor.matmul(
                                        po[hh, c][:],
                                        vh[tk][:, head * (D + 1) : (head + 1) * (D + 1)],
                                        es[hh][:, c * 512 : (c + 1) * 512],
                                        start=(tk == 0),
                                        stop=(tk == LT - 1),
                                    )
                        for hh in range(2):
                            for c in range(2):
                                lo = hf * TQH + c * 512
                                nc.vector.tensor_copy(av[hh][:, lo : lo + 512], po[hh, c][:])
                    for hh in range(2):
                        head = 2 * p + hh
                        nc.sync.dma_start(outT[head * D : (head + 1) * D, :], av[hh][0:D, :])
                        nc.sync.dma_start(sums[head : head + 1, :], av[hh][D : D + 1, :])

    nc.compile()
    return nc


_program_cache = {}


def _get_program(mm_dt=MM_DT):
    key = str(mm_dt)
    if key not in _program_cache:
        _program_cache[key] = build_program(mm_dt)
    return _program_cache[key]


def _make_in_maps(q, k, v, Wq, Wk, Wv):
    scale = 1.0 / np.sqrt(np.float32(D))
    q = np.asarray(q, dtype=np.float32)
    k = np.asarray(k, dtype=np.float32)
    v = np.asarray(v, dtype=np.float32)
    xqs = [np.ascontiguousarray(q[b].T) for b in range(B)]
    xks = [np.ascontiguousarray(k[b].T) for b in range(B)]
    xvs = [np.ascontiguousarray(v[b].T) for b in range(B)]
    in_maps = []
    for c in range(NCORES):
        b = c // 2
        h0 = (c % 2) * H
        rows = slice(h0 * D, (h0 + H) * D)
        in_maps.append(
            {
                "xq": xqs[b],
                "xk": xks[b],
                "xv": xvs[b],
                "wq": np.ascontiguousarray(np.asarray(Wq, np.float32)[rows, :].T * scale),
                "wk": np.ascontiguousarray(np.asarray(Wk, np.float32)[rows, :].T),
                "wv": np.ascontiguousarray(np.asarray(Wv, np.float32)[rows, :].T),
            }
        )
    return in_maps


def _assemble(results):
    out = np.empty((B, L, NH * D), dtype=np.float32)
    for c in range(NCORES):
        b = c // 2
        h0 = (c % 2) * H
        r = results[c]
        unnorm = r["outT"].reshape(H, D, L)
        denom = r["sums"].reshape(H, 1, L)
        core_out = (unnorm / denom).transpose(2, 0, 1).reshape(L, F)
        out[b, :, h0 * D : (h0 + H) * D] = core_out
    return out


def run_mha(q, k, v, Wq, Wk, Wv, trace=False, mm_dt=MM_DT, **run_kwargs):
    nc = _get_program(mm_dt)
    in_maps = _make_in_maps(q, k, v, Wq, Wk, Wv)
    res = run_bass_kernel_spmd(nc, in_maps, core_ids=list(range(NCORES)), trace=trace, **run_kwargs)
    return _assemble(res.results), res


def kernel(q, k, v, Wq, Wk, Wv):
    out, _ = run_mha(q, k, v, Wq, Wk, Wv)
    return out
